# revision 33
# baseline (speedup 1.0000x reference)
"""LoRA QKV fused projection kernel for 8 TRN2 NeuronCores.

Reference computation (T=8192 tokens, HID=4096, D=6144 out, S=8 slots, R=16):
    y = x @ W.T
    a[t,s,i,r] = sum_h x[t,h] * lora_A[s,i,r,h]         (down-proj, all slots)
    a *= onehot(token_to_slot)[t,s] * scaling[s]         (routing gate)
    d[t, :] = concat_i( sum_{s,r} a[t,s,i,r] * B_i[s,:,r] )   (up-proj)
    out = y + d

Sharding: pure data-parallel over tokens. Core c owns tokens
[c*1024, (c+1)*1024) and computes its full [6144, 1024] output column
block; host assembly is a transpose-concat (no reduction).

Per-core dataflow (all matmuls bf16 x bf16 -> fp32 PSUM, 1 col/cycle):
  * x shard resident in SBUF as [128(k-part), 32(k-tile), 1024(tok)] bf16,
    streamed in 4-ktile chunks so phase A starts immediately.
  * Phase A: LoRA down-proj aT[(i,sr), t] = A^T x, PSUM-accumulated over
    all 32 k-tiles into 6 psum tiles (3 targets x 2 token halves).
  * Phase B: routing gate (host-built onehot*scaling, expanded over rank)
    applied on DVE: ag = aT * gate, written as bf16.
  * Phase C: per output row-block mb (48): 32-step K accumulation of
    W[mb] @ x into 2 psum tiles (token halves), then the LoRA up-proj
    B[mb] @ ag[i] is accumulated INTO THE SAME psum (start=False), so the
    base projection and LoRA delta fuse for free; one copy + one DMA out.

bf16 keeps the PE at full rate (fp32r is also 1 col/cycle but stalls on
long single-bank accumulation chains and doubles LDWEIGHTS bytes), cuts
W streaming to 50 MB/core, and lands ~2.4e-3 rel err (gate 2e-2).
"""

import numpy as np
import ml_dtypes

# problem shape (hardcoded per harness contract)
T = 8192
HID = 4096
Q_SIZE = 4096
KV_SIZE = 1024
D = Q_SIZE + 2 * KV_SIZE  # 6144
S = 8
R = 16
NCORES = 8
P = 128

TC = T // NCORES          # 1024 tokens per core
MB = D // P               # 48 output row-blocks of 128
KA = HID // P             # 32 k-tiles
KCH = 4                   # k-tiles per streamed x chunk
NH = TC // 512            # 2 token halves (psum bank = 512 fp32)

_CACHE = {}


def _build_nc():
    import concourse.mybir as mybir
    import concourse.tile as tile
    from concourse import bacc

    bf16 = mybir.dt.bfloat16
    f32 = mybir.dt.float32
    f8 = mybir.dt.float8e4

    nc = bacc.Bacc(None, target_bir_lowering=False, debug=False)

    # ---- DRAM parameters (per-core shapes)
    x_d = nc.declare_dram_parameter("x_sh", [P, KA, TC], bf16, isOutput=False)
    w_d = nc.declare_dram_parameter("w_t", [MB, P, KA, P], bf16, isOutput=False)
    a8_d = nc.declare_dram_parameter("a8", [P, KA // 2, 2, 3 * P], f8, isOutput=False)
    b_d = nc.declare_dram_parameter("b_t", [P, MB, P], bf16, isOutput=False)
    g_d = nc.declare_dram_parameter("gate", [P, TC], f32, isOutput=False)
    y_d = nc.declare_dram_parameter("y_out", [MB, P, TC], f32, isOutput=True)

    with tile.TileContext(nc) as tc:
        with tc.tile_pool(name="xres", bufs=1) as xres_pool, \
             tc.tile_pool(name="wp", bufs=3) as w_pool, \
             tc.tile_pool(name="ab", bufs=1) as ab_pool, \
             tc.tile_pool(name="agp", bufs=1) as ag_pool, \
             tc.tile_pool(name="stp", bufs=3) as st_pool, \
             tc.tile_pool(name="psum", bufs=8, space="PSUM") as ps_pool:

            # resident operands
            x_res = xres_pool.tile([P, KA, TC], bf16, tag="xres")
            x8_t = ab_pool.tile([P, KA // 2, 2, TC], f8, tag="x8")
            a8_t = ab_pool.tile([P, KA // 2, 2, 3 * P], f8, tag="a8")
            b_t = ab_pool.tile([P, MB, P], bf16, tag="b")
            gate_t = ab_pool.tile([P, TC], f32, tag="gate")

            # ---------------- Phase A: LoRA down-proj aT = A @ x ------------
            # fp8 e4m3 DoubleRow: each instruction contracts a k-tile PAIR
            # (256 hidden dims), halving PE time vs bf16. The fp8 x copy is
            # derived ON-DEVICE from the streaming bf16 x (cast copies that
            # alternate between the idle DVE and Act engines), so only the
            # tiny fp8 A table is an extra DMA. Host pre-scales A by SA;
            # 1/SA is folded into the gate.
            nc.scalar.dma_start(out=a8_t[:], in_=a8_d[:])
            ps_a = [
                ps_pool.tile([P, 512], f32, tag="ps", name=f"ps_a{i}_{h}")
                for i in range(3) for h in range(NH)
            ]
            JA = KA // 2
            for ch in range(KA // KCH):
                nc.sync.dma_start(
                    out=x_res[:, ch * KCH:(ch + 1) * KCH, :],
                    in_=x_d[:, ch * KCH:(ch + 1) * KCH, :],
                )
                for kk in range(KCH):
                    k = ch * KCH + kk
                    j, two = divmod(k, 2)
                    eng = nc.vector.tensor_copy if k % 2 == 0 else nc.scalar.copy
                    eng(x8_t[:, j, two, :], x_res[:, k, :])
                for jj in range(KCH // 2):
                    j = (ch * KCH) // 2 + jj
                    for i in range(3):
                        for h in range(NH):
                            nc.tensor.matmul(
                                ps_a[i * NH + h][:],
                                a8_t[:, j, :, i * P:(i + 1) * P],
                                x8_t[:, j, :, h * 512:(h + 1) * 512],
                                start=(j == 0), stop=(j == JA - 1),
                                perf_mode=mybir.MatmulPerfMode.DoubleRow,
                            )
            nc.sync.dma_start(out=gate_t[:], in_=g_d[:])
            nc.sync.dma_start(out=b_t[:], in_=b_d[:])

            # ---------------- Phase B: routing gate ------------------------
            ag = []
            for i in range(3):
                ag_t = ag_pool.tile([P, TC], bf16, tag=f"ag{i}", name=f"ag{i}")
                for h in range(NH):
                    sl = slice(h * 512, (h + 1) * 512)
                    nc.vector.tensor_mul(ag_t[:, sl], ps_a[i * NH + h][:], gate_t[:, sl])
                ag.append(ag_t)

            # ------------- Phase C: main GEMM + fused LoRA up-proj ----------
            for mb in range(MB):
                w_t = w_pool.tile([P, KA, P], bf16, tag="w", name=f"w{mb}")
                nc.scalar.dma_start(out=w_t[:], in_=w_d[mb])
                i = 0 if mb < Q_SIZE // P else (1 if mb < (Q_SIZE + KV_SIZE) // P else 2)
                pss = [
                    ps_pool.tile([P, 512], f32, tag="ps", name=f"pm{mb}_{h}")
                    for h in range(NH)
                ]
                for k in range(KA):
                    for h in range(NH):
                        nc.tensor.matmul(
                            pss[h][:],
                            w_t[:, k, :],
                            x_res[:, k, h * 512:(h + 1) * 512],
                            start=(k == 0), stop=False,
                        )
                st = st_pool.tile([P, TC], f32, tag="st", name=f"st{mb}")
                for h in range(NH):
                    nc.tensor.matmul(
                        pss[h][:],
                        b_t[:, mb, :],
                        ag[i][:, h * 512:(h + 1) * 512],
                        start=False, stop=True,
                    )
                    nc.vector.tensor_copy(
                        st[:, h * 512:(h + 1) * 512], pss[h][:])
                    nc.sync.dma_start(
                        out=y_d[mb, :, h * 512:(h + 1) * 512],
                        in_=st[:, h * 512:(h + 1) * 512],
                    )

    nc.compile()
    return nc


def _get_nc():
    if "nc" not in _CACHE:
        _CACHE["nc"] = _build_nc()
    return _CACHE["nc"]


def _prep_in_maps(x, W, lora_A, lora_B_q, lora_B_k, lora_B_v, scaling, token_to_slot):
    f = np.float32
    bf = ml_dtypes.bfloat16
    x = np.ascontiguousarray(x, dtype=f)
    W = np.ascontiguousarray(W, dtype=f)

    # x shard, moving operand: [c, p, ka, tl]  (h = ka*128 + p, t = c*1024 + tl)
    x_f32 = np.ascontiguousarray(
        x.reshape(NCORES, TC, KA, P).transpose(0, 3, 2, 1))
    x_sh = x_f32.astype(bf)
    # W stationary: [mb, p, ka, dl]  (d = mb*128 + dl)  -- replicated
    w_t = np.ascontiguousarray(
        W.reshape(MB, P, KA, P).transpose(0, 3, 2, 1)).astype(bf)
    # fp8 e4m3 LoRA A table (DoubleRow pairs of k-tiles); the fp8 x copy is
    # derived on-device. The 1/SA descale folds into the gate below.
    SA = np.float32(1024.0)
    f8 = ml_dtypes.float8_e4m3
    a_f32 = np.ascontiguousarray(
        np.asarray(lora_A, dtype=f).reshape(S, 3, R, KA, P).transpose(4, 3, 1, 0, 2)
        .reshape(P, KA, 3, S * R))
    a8 = np.ascontiguousarray(
        (a_f32 * SA).astype(f8).reshape(P, KA // 2, 2, 3 * S * R))
    # LoRA B stationary: [(s r), mb, dl] -- replicated
    bq = np.asarray(lora_B_q, dtype=f).transpose(0, 2, 1).reshape(S * R, Q_SIZE)
    bk = np.asarray(lora_B_k, dtype=f).transpose(0, 2, 1).reshape(S * R, KV_SIZE)
    bv = np.asarray(lora_B_v, dtype=f).transpose(0, 2, 1).reshape(S * R, KV_SIZE)
    b_t = np.ascontiguousarray(
        np.concatenate([bq, bk, bv], axis=1).reshape(S * R, MB, P)).astype(bf)
    # routing gate, expanded over ranks: [c, (s r), tl]; carries the fp8
    # descale 1/(SX*SA) so the device never sees the scales.
    slot = np.asarray(token_to_slot).reshape(NCORES, TC)
    g = (slot[:, None, :] == np.arange(S, dtype=slot.dtype)[None, :, None])
    g = g.astype(f) * (np.asarray(scaling, dtype=f) / SA)[None, :, None]
    gate = np.ascontiguousarray(np.repeat(g, R, axis=1))

    in_maps = []
    for c in range(NCORES):
        in_maps.append({
            "x_sh": x_sh[c],
            "w_t": w_t,
            "a8": a8,
            "b_t": b_t,
            "gate": gate[c],
        })
    return in_maps


def _assemble(results):
    out = np.empty((T, D), dtype=np.float32)
    for c in range(NCORES):
        out[c * TC:(c + 1) * TC, :] = results[c]["y_out"].reshape(D, TC).T
    return out


def _run(inputs, trace=False):
    from concourse.bass_utils import run_bass_kernel_spmd
    nc = _get_nc()
    in_maps = _prep_in_maps(**inputs)
    res = run_bass_kernel_spmd(
        nc, in_maps, core_ids=list(range(NCORES)), trace=trace)
    return res


def kernel(**inputs) -> np.ndarray:
    res = _run(inputs, trace=False)
    return _assemble(res.results)


if __name__ == "__main__":
    rng = np.random.default_rng(0)
    ins = {
        "x": rng.standard_normal((T, HID)).astype(np.float32),
        "W": (rng.standard_normal((D, HID)) * 0.02).astype(np.float32),
        "lora_A": (rng.standard_normal((S, 3, R, HID)) * 0.02).astype(np.float32),
        "lora_B_q": (rng.standard_normal((S, Q_SIZE, R)) * 0.02).astype(np.float32),
        "lora_B_k": (rng.standard_normal((S, KV_SIZE, R)) * 0.02).astype(np.float32),
        "lora_B_v": (rng.standard_normal((S, KV_SIZE, R)) * 0.02).astype(np.float32),
        "scaling": rng.uniform(0.5, 2.0, S).astype(np.float32),
        "token_to_slot": rng.integers(0, S, T).astype(np.int32),
    }
    out = kernel(**ins)
    print("out", out.shape, out.dtype)


# revision 34
# speedup vs baseline: 1.1729x; 1.1729x over previous
"""LoRA QKV fused projection kernel for 8 TRN2 NeuronCores.

Reference computation (T=8192 tokens, HID=4096, D=6144 out, S=8 slots, R=16):
    y = x @ W.T
    a[t,s,i,r] = sum_h x[t,h] * lora_A[s,i,r,h]         (down-proj, all slots)
    a *= onehot(token_to_slot)[t,s] * scaling[s]         (routing gate)
    d[t, :] = concat_i( sum_{s,r} a[t,s,i,r] * B_i[s,:,r] )   (up-proj)
    out = y + d

Sharding: pure data-parallel over tokens. Core c owns tokens
[c*1024, (c+1)*1024) and computes its full [6144, 1024] output column
block; host assembly is a transpose-concat (no reduction).

Per-core dataflow (all matmuls bf16 x bf16 -> fp32 PSUM, 1 col/cycle):
  * x shard resident in SBUF as [128(k-part), 32(k-tile), 1024(tok)] bf16,
    streamed in 4-ktile chunks so phase A starts immediately.
  * Phase A: LoRA down-proj aT[(i,sr), t] = A^T x, PSUM-accumulated over
    all 32 k-tiles into 6 psum tiles (3 targets x 2 token halves).
  * Phase B: routing gate (host-built onehot*scaling, expanded over rank)
    applied on DVE: ag = aT * gate, written as bf16.
  * Phase C: per output row-block mb (48): 32-step K accumulation of
    W[mb] @ x into 2 psum tiles (token halves), then the LoRA up-proj
    B[mb] @ ag[i] is accumulated INTO THE SAME psum (start=False), so the
    base projection and LoRA delta fuse for free; one copy + one DMA out.

bf16 keeps the PE at full rate (fp32r is also 1 col/cycle but stalls on
long single-bank accumulation chains and doubles LDWEIGHTS bytes), cuts
W streaming to 50 MB/core, and lands ~2.4e-3 rel err (gate 2e-2).
"""

import numpy as np
import ml_dtypes

# problem shape (hardcoded per harness contract)
T = 8192
HID = 4096
Q_SIZE = 4096
KV_SIZE = 1024
D = Q_SIZE + 2 * KV_SIZE  # 6144
S = 8
R = 16
NCORES = 8
P = 128

TC = T // NCORES          # 1024 tokens per core
MB = D // P               # 48 output row-blocks of 128
KA = HID // P             # 32 k-tiles
KCH = 4                   # k-tiles per streamed x chunk
NH = TC // 512            # 2 token halves (psum bank = 512 fp32)

_CACHE = {}


def _build_nc():
    import concourse.mybir as mybir
    import concourse.tile as tile
    from concourse import bacc

    bf16 = mybir.dt.bfloat16
    f32 = mybir.dt.float32
    f8 = mybir.dt.float8e4

    nc = bacc.Bacc(None, target_bir_lowering=False, debug=False)

    # ---- DRAM parameters (per-core shapes)
    PAX = 3 * P + TC  # per-(j,pair) packed row: 3 A targets then x tokens
    x_d = nc.declare_dram_parameter("x_sh", [P, KA, TC], bf16, isOutput=False)
    w_d = nc.declare_dram_parameter("w_t", [MB, P, KA, P], bf16, isOutput=False)
    pax_d = nc.declare_dram_parameter("pax", [P, KA // 2, 2, PAX], f8, isOutput=False)
    b_d = nc.declare_dram_parameter("b_t", [P, MB, P], bf16, isOutput=False)
    g_d = nc.declare_dram_parameter("gate", [P, TC], f32, isOutput=False)
    y_d = nc.declare_dram_parameter("y_out", [MB, P, TC], f32, isOutput=True)

    with tile.TileContext(nc) as tc:
        with tc.tile_pool(name="xres", bufs=1) as xres_pool, \
             tc.tile_pool(name="wp", bufs=3) as w_pool, \
             tc.tile_pool(name="ab", bufs=1) as ab_pool, \
             tc.tile_pool(name="agp", bufs=1) as ag_pool, \
             tc.tile_pool(name="stp", bufs=3) as st_pool, \
             tc.tile_pool(name="psum", bufs=8, space="PSUM") as ps_pool:

            # resident operands
            x_res = xres_pool.tile([P, KA, TC], bf16, tag="xres")
            pax_t = ab_pool.tile([P, KA // 2, 2, PAX], f8, tag="pax")
            b_t = ab_pool.tile([P, MB, P], bf16, tag="b")
            gate_t = ab_pool.tile([P, TC], f32, tag="gate")

            # x (bf16, for the main GEMM) streams on the sync queue; the fp8
            # LoRA pack + W stream on the scalar queue in parallel.
            for ch in range(KA // KCH):
                nc.sync.dma_start(
                    out=x_res[:, ch * KCH:(ch + 1) * KCH, :],
                    in_=x_d[:, ch * KCH:(ch + 1) * KCH, :],
                )

            # ---------------- Phase A: LoRA down-proj aT = A @ x ------------
            # fp8 e4m3 DoubleRow: each instruction contracts a k-tile PAIR
            # (256 hidden dims), halving PE time vs bf16. Host pre-scales
            # x by SX and A by SA; 1/(SX*SA) is folded into the gate.
            ps_a = [
                ps_pool.tile([P, 512], f32, tag="ps", name=f"ps_a{i}_{h}")
                for i in range(3) for h in range(NH)
            ]
            JA = KA // 2
            jch = [1, 1, 2, 4, 4, 4]  # j-tiles per streamed chunk
            j0 = 0
            for jc in jch:
                nc.scalar.dma_start(
                    out=pax_t[:, j0:j0 + jc], in_=pax_d[:, j0:j0 + jc])
                for j in range(j0, j0 + jc):
                    for i in range(3):
                        for h in range(NH):
                            nc.tensor.matmul(
                                ps_a[i * NH + h][:],
                                pax_t[:, j, :, i * P:(i + 1) * P],
                                pax_t[:, j, :, 3 * P + h * 512:3 * P + (h + 1) * 512],
                                start=(j == 0), stop=(j == JA - 1),
                                perf_mode=mybir.MatmulPerfMode.DoubleRow,
                            )
                j0 += jc
            nc.sync.dma_start(out=gate_t[:], in_=g_d[:])
            nc.sync.dma_start(out=b_t[:], in_=b_d[:])

            # ---------------- Phase B: routing gate ------------------------
            ag = []
            for i in range(3):
                ag_t = ag_pool.tile([P, TC], bf16, tag=f"ag{i}", name=f"ag{i}")
                for h in range(NH):
                    sl = slice(h * 512, (h + 1) * 512)
                    nc.vector.tensor_mul(ag_t[:, sl], ps_a[i * NH + h][:], gate_t[:, sl])
                ag.append(ag_t)

            # ------------- Phase C: main GEMM + fused LoRA up-proj ----------
            for mb in range(MB):
                w_t = w_pool.tile([P, KA, P], bf16, tag="w", name=f"w{mb}")
                nc.scalar.dma_start(out=w_t[:], in_=w_d[mb])
                i = 0 if mb < Q_SIZE // P else (1 if mb < (Q_SIZE + KV_SIZE) // P else 2)
                pss = [
                    ps_pool.tile([P, 512], f32, tag="ps", name=f"pm{mb}_{h}")
                    for h in range(NH)
                ]
                for k in range(KA):
                    for h in range(NH):
                        nc.tensor.matmul(
                            pss[h][:],
                            w_t[:, k, :],
                            x_res[:, k, h * 512:(h + 1) * 512],
                            start=(k == 0), stop=False,
                        )
                st = st_pool.tile([P, TC], f32, tag="st", name=f"st{mb}")
                for h in range(NH):
                    nc.tensor.matmul(
                        pss[h][:],
                        b_t[:, mb, :],
                        ag[i][:, h * 512:(h + 1) * 512],
                        start=False, stop=True,
                    )
                    nc.vector.tensor_copy(
                        st[:, h * 512:(h + 1) * 512], pss[h][:])
                    nc.sync.dma_start(
                        out=y_d[mb, :, h * 512:(h + 1) * 512],
                        in_=st[:, h * 512:(h + 1) * 512],
                    )

    nc.compile()
    return nc


def _get_nc():
    if "nc" not in _CACHE:
        _CACHE["nc"] = _build_nc()
    return _CACHE["nc"]


def _prep_in_maps(x, W, lora_A, lora_B_q, lora_B_k, lora_B_v, scaling, token_to_slot):
    f = np.float32
    bf = ml_dtypes.bfloat16
    x = np.ascontiguousarray(x, dtype=f)
    W = np.ascontiguousarray(W, dtype=f)

    # x shard, moving operand: [c, p, ka, tl]  (h = ka*128 + p, t = c*1024 + tl)
    x_f32 = np.ascontiguousarray(
        x.reshape(NCORES, TC, KA, P).transpose(0, 3, 2, 1))
    x_sh = x_f32.astype(bf)
    # W stationary: [mb, p, ka, dl]  (d = mb*128 + dl)  -- replicated
    w_t = np.ascontiguousarray(
        W.reshape(MB, P, KA, P).transpose(0, 3, 2, 1)).astype(bf)
    # fp8 e4m3 copies for the LoRA down-proj (DoubleRow pairs of k-tiles),
    # packed [A targets | x tokens] per (j, pair) row so each chunk is one
    # DMA; the 1/(SX*SA) descale folds into the gate below.
    SX, SA = np.float32(32.0), np.float32(1024.0)
    f8 = ml_dtypes.float8_e4m3
    x8 = (x_f32 * SX).astype(f8).reshape(NCORES, P, KA // 2, 2, TC)
    a_f32 = np.ascontiguousarray(
        np.asarray(lora_A, dtype=f).reshape(S, 3, R, KA, P).transpose(4, 3, 1, 0, 2)
        .reshape(P, KA, 3, S * R))
    a8 = (a_f32 * SA).astype(f8).reshape(P, KA // 2, 2, 3 * S * R)
    pax = np.concatenate(
        [np.broadcast_to(a8, (NCORES,) + a8.shape), x8], axis=-1)
    pax = np.ascontiguousarray(pax)
    # LoRA B stationary: [(s r), mb, dl] -- replicated
    bq = np.asarray(lora_B_q, dtype=f).transpose(0, 2, 1).reshape(S * R, Q_SIZE)
    bk = np.asarray(lora_B_k, dtype=f).transpose(0, 2, 1).reshape(S * R, KV_SIZE)
    bv = np.asarray(lora_B_v, dtype=f).transpose(0, 2, 1).reshape(S * R, KV_SIZE)
    b_t = np.ascontiguousarray(
        np.concatenate([bq, bk, bv], axis=1).reshape(S * R, MB, P)).astype(bf)
    # routing gate, expanded over ranks: [c, (s r), tl]; carries the fp8
    # descale 1/(SX*SA) so the device never sees the scales.
    slot = np.asarray(token_to_slot).reshape(NCORES, TC)
    g = (slot[:, None, :] == np.arange(S, dtype=slot.dtype)[None, :, None])
    g = g.astype(f) * (np.asarray(scaling, dtype=f) / (SX * SA))[None, :, None]
    gate = np.ascontiguousarray(np.repeat(g, R, axis=1))

    in_maps = []
    for c in range(NCORES):
        in_maps.append({
            "x_sh": x_sh[c],
            "w_t": w_t,
            "pax": pax[c],
            "b_t": b_t,
            "gate": gate[c],
        })
    return in_maps


def _assemble(results):
    out = np.empty((T, D), dtype=np.float32)
    for c in range(NCORES):
        out[c * TC:(c + 1) * TC, :] = results[c]["y_out"].reshape(D, TC).T
    return out


def _run(inputs, trace=False):
    from concourse.bass_utils import run_bass_kernel_spmd
    nc = _get_nc()
    in_maps = _prep_in_maps(**inputs)
    res = run_bass_kernel_spmd(
        nc, in_maps, core_ids=list(range(NCORES)), trace=trace)
    return res


def kernel(**inputs) -> np.ndarray:
    res = _run(inputs, trace=False)
    return _assemble(res.results)


if __name__ == "__main__":
    rng = np.random.default_rng(0)
    ins = {
        "x": rng.standard_normal((T, HID)).astype(np.float32),
        "W": (rng.standard_normal((D, HID)) * 0.02).astype(np.float32),
        "lora_A": (rng.standard_normal((S, 3, R, HID)) * 0.02).astype(np.float32),
        "lora_B_q": (rng.standard_normal((S, Q_SIZE, R)) * 0.02).astype(np.float32),
        "lora_B_k": (rng.standard_normal((S, KV_SIZE, R)) * 0.02).astype(np.float32),
        "lora_B_v": (rng.standard_normal((S, KV_SIZE, R)) * 0.02).astype(np.float32),
        "scaling": rng.uniform(0.5, 2.0, S).astype(np.float32),
        "token_to_slot": rng.integers(0, S, T).astype(np.int32),
    }
    out = kernel(**ins)
    print("out", out.shape, out.dtype)
